# revision 3
# baseline (speedup 1.0000x reference)
"""Trainium2 Bass kernel for nn_ConcatMLPUpdate (gnn_message_passing).

Strategy (8 NeuronCores, SPMD):
  - Bin-pack the 32 batch rows into 8 groups of 4 so each core owns ~256
    mentions (scatter-add back into encoded_input stays core-local); the
    output rows are un-permuted on the host.
  - All heavy activations are feature-major ([features<=128 on partitions,
    rows on the free dim]) so every matmul consumes natural-layout weights as
    the stationary operand and no transposes appear in the hot loop.
  - The three large matmuls (cd, ac1, ac2) run in fp8e4m3 with DoubleRow
    perf mode (2 contraction tiles per pass -> 2x PE rate). Weights are
    pre-scaled by a power of two on the host; the dequant rides the ACT/DVE
    evacuation scale.
  - The per-mention projection term enters the cm PSUM accumulation via a
    k-broadcast matmul of pmv (no per-chunk DVE broadcast add, no T1 buffer).
  - LayerNorm over the feature (partition) dim uses ones-vector matmuls for
    sum/sum-of-squares and a rank-1 ones matmul to broadcast per-row stats
    back across partitions.
  - The scatter-add is a one-hot matmul (handles duplicate target slots
    exactly), fused with the final row-major LayerNorm pass.

kernel(**inputs) takes the full unsharded inputs and returns the full output.
"""

import math
import os
import sys

import numpy as np

for _p in ("/opt/trn_rl_repo", "/root/.axon_site/_ro/trn_rl_repo"):
    if os.path.isdir(_p) and _p not in sys.path:
        sys.path.append(_p)

import concourse.bass as bass
from concourse import bacc
import concourse.tile as tile
from concourse import mybir
from concourse.bass import IndirectOffsetOnAxis
from concourse.bass_utils import run_bass_kernel_spmd
from concourse.masks import make_identity

# problem constants
B, T, D = 32, 512, 768
M, K, R = 2048, 32, 128
H = 1024
EPS = 1e-12
NCORES = 8
BPC = B // NCORES            # batch rows per core
TOK = BPC * T                # token slots per core
P = 128
CH = 16                      # mentions per main-loop chunk
NF = CH * K                  # free-dim columns per chunk (512)
DB = D // P                  # 6 feature blocks of D
HB = H // P                  # 8 feature blocks of H

F32 = mybir.dt.float32
F32R = mybir.dt.float32r
BF16 = mybir.dt.bfloat16
FP8 = mybir.dt.float8e4
I32 = mybir.dt.int32
AF = mybir.ActivationFunctionType
OP = mybir.AluOpType
AX = mybir.AxisListType
DR = mybir.MatmulPerfMode.DoubleRow


def _build(mc_pad, scales, use_f32r=True, use_tanh_gelu=True, use_bf16=True,
           use_fp8=True, aclb_zero=True, b2_zero=True, pl_triv=True,
           ln_triv=True):
    """Build the Bass program for a padded per-core mention count."""
    NCH = mc_pad // CH
    NMT = (mc_pad + P - 1) // P      # 128-mention blocks (gather/scatter)
    OHR = NMT * P                    # one-hot row count (mc_pad padded to 128)
    NR = mc_pad * K                  # retrieval rows per core
    NTT = TOK // P                   # token tiles (16)
    GELU = AF.Gelu_apprx_tanh if use_tanh_gelu else AF.Gelu
    s_cd, s_a1, s_a2 = scales

    nc = bacc.Bacc("TRN2", target_bir_lowering=False, debug=False)

    def rr(ap):
        if ap.dtype != F32:
            return ap
        return ap.bitcast(F32R) if use_f32r else ap

    def ro(ap):
        # producer out-AP cast: ACT/DVE round their output to f32r precision
        return ap.bitcast(F32R) if use_f32r else ap

    WDT = BF16 if use_bf16 else (F32R if use_f32r else F32)
    ADT = BF16 if use_bf16 else F32
    MDT = FP8 if use_fp8 else WDT    # dtype of the three big weight sets

    def ra(ap):
        # round/cast for matmul-input activations produced by ACT/DVE
        if use_bf16:
            return ap
        return ro(ap)

    # ---- I/O ----
    dp = nc.declare_dram_parameter
    enc_h = dp("enc", [TOK, D], F32, isOutput=False)
    rvt_h = dp("rvt", [R, NR], WDT, isOutput=False)
    sco_h = dp("sco", [1, NR], F32, isOutput=False)
    msk_h = dp("msk", [1, mc_pad], WDT, isOutput=False)
    mskp_h = dp("mskp", [OHR], F32, isOutput=False)
    gis_h = dp("gis", [OHR], I32, isOutput=False)
    gie_h = dp("gie", [OHR], I32, isOutput=False)
    oh_h = dp("oh", [OHR, TOK], WDT, isOutput=False)
    vpw_h = dp("vpw", [2 * D, R], WDT, isOutput=False)
    vpb_h = dp("vpb", [P, 1], F32, isOutput=False)
    cmw_h = dp("cmw", [2 * R, H], WDT, isOutput=False)
    cmb_h = dp("cmb", [P, HB], F32, isOutput=False)
    cdw_h = dp("cdw", [H, D], MDT, isOutput=False)
    cdb_h = dp("cdb", [P, DB], F32, isOutput=False)
    a1w_h = dp("a1w", [D, H], MDT, isOutput=False)
    a1b_h = dp("a1b", [P, HB], F32, isOutput=False)
    a2w_h = dp("a2w", [H, D], MDT, isOutput=False)
    a2b_h = dp("a2b", [P, DB], F32, isOutput=False)
    acls_h = dp("acls", [P, DB], F32, isOutput=False)
    aclb_h = dp("aclb", [P, DB], F32, isOutput=False)
    p1w_h = dp("p1w", [D, H], WDT, isOutput=False)
    p1b_h = dp("p1b", [P, HB], F32, isOutput=False)
    p2w_h = dp("p2w", [H, D], WDT, isOutput=False)
    p2b_h = dp("p2b", [P, DB], F32, isOutput=False)
    plls_h = dp("plls", [P, DB], F32, isOutput=False)
    pllb_h = dp("pllb", [P, DB], F32, isOutput=False)
    lns_h = dp("lns", [1, D], F32, isOutput=False)
    lnb_h = dp("lnb", [1, D], F32, isOutput=False)
    out_h = dp("out", [TOK, D], F32, isOutput=True)

    with tile.TileContext(nc) as tc:
        with (
            tc.tile_pool(name="const", bufs=1) as const,
            tc.tile_pool(name="w512", bufs=2) as w512,
            tc.tile_pool(name="w768", bufs=7) as w768,
            tc.tile_pool(name="rvp", bufs=5) as rvp,
            tc.tile_pool(name="wblk", bufs=4) as wblk,
            tc.tile_pool(name="act8", bufs=2) as act8,
            tc.tile_pool(name="sm", bufs=2) as sm,
            tc.tile_pool(name="pacc", bufs=4 if aclb_zero else 3, space="PSUM") as pacc,
            tc.tile_pool(name="pb", bufs=2 if aclb_zero else 3, space="PSUM") as pb,
            tc.tile_pool(name="pst", bufs=1, space="PSUM") as pst,
        ):
            # ---- index DMAs + endpoint gathers first (long pole) ----
            gis_sb = const.tile([P, NMT], I32, name="gis_sb")
            nc.sync.dma_start(gis_sb[:], gis_h[:].rearrange("(t p) -> p t", p=P))
            gie_sb = const.tile([P, NMT], I32, name="gie_sb")
            nc.sync.dma_start(gie_sb[:], gie_h[:].rearrange("(t p) -> p t", p=P))
            gx = []
            for t in range(NMT):
                xs = w768.tile([P, D], F32, tag="g768", name=f"xs{t}")
                nc.gpsimd.indirect_dma_start(
                    out=xs[:], out_offset=None, in_=enc_h[:],
                    in_offset=IndirectOffsetOnAxis(ap=gis_sb[:, t:t + 1], axis=0))
                xe = w768.tile([P, D], F32, tag="g768", name=f"xe{t}")
                nc.gpsimd.indirect_dma_start(
                    out=xe[:], out_offset=None, in_=enc_h[:],
                    in_offset=IndirectOffsetOnAxis(ap=gie_sb[:, t:t + 1], axis=0))
                gx.append((xs, xe))

            # ---- persistent tiles (ordered so early-needed data lands first) ----
            w_vp = const.tile([P, 12, P], WDT, name="w_vp")
            nc.sync.dma_start(w_vp[:], vpw_h[:].rearrange("(kb p) m -> p kb m", p=P))
            w_cm = const.tile([P, 2, H], WDT, name="w_cm")
            nc.sync.dma_start(w_cm[:], cmw_h[:].rearrange("(kb p) m -> p kb m", p=P))
            NPRE = min(4, NCH)
            rv_pre = []
            for c in range(NPRE):
                rv_t = rvp.tile([P, NF], WDT, tag="rv", bufs=5, name=f"rvp{c}")
                nc.sync.dma_start(rv_t[:], rvt_h[:, c * NF:(c + 1) * NF])
                rv_pre.append(rv_t)
            def bias_tile(h, cols, nm):
                t = const.tile([P, cols], F32, name=nm)
                nc.sync.dma_start(t[:], h[:])
                return t

            vp_b = bias_tile(vpb_h, 1, "vp_b")
            cm_b = bias_tile(cmb_h, HB, "cm_b")
            cd_b = bias_tile(cdb_h, DB, "cd_b")
            a1_b = bias_tile(a1b_h, HB, "a1_b")
            a2_b = bias_tile(a2b_h, DB, "a2_b")
            ac_ls = bias_tile(acls_h, DB, "ac_ls")
            ac_lb = bias_tile(aclb_h, DB, "ac_lb")
            p1_b = bias_tile(p1b_h, HB, "p1_b")
            p2_b = bias_tile(p2b_h, DB, "p2_b")
            pl_ls = bias_tile(plls_h, DB, "pl_ls")
            pl_lb = bias_tile(pllb_h, DB, "pl_lb")

            w_cd = const.tile([P, HB, D], MDT, name="w_cd")
            nc.sync.dma_start(w_cd[:], cdw_h[:].rearrange("(kb p) m -> p kb m", p=P))
            w_a1 = const.tile([P, DB, H], MDT, name="w_a1")
            nc.sync.dma_start(w_a1[:], a1w_h[:].rearrange("(kb p) m -> p kb m", p=P))
            w_a2 = const.tile([P, HB, D], MDT, name="w_a2")
            nc.sync.dma_start(w_a2[:], a2w_h[:].rearrange("(kb p) m -> p kb m", p=P))

            if not ln_triv:
                lns_rep = const.tile([P, D], F32, name="lns_rep")
                nc.sync.dma_start(lns_rep[:], lns_h[:].to_broadcast((P, D)))
                lnb_rep = const.tile([P, D], F32, name="lnb_rep")
                nc.sync.dma_start(lnb_rep[:], lnb_h[:].to_broadcast((P, D)))

            msk_sb = const.tile([1, mc_pad], WDT, name="msk_sb")
            nc.sync.dma_start(msk_sb[:], msk_h[:])

            ident = const.tile([P, P], F32, name="ident")
            make_identity(nc, ident[:])
            if ADT != F32:
                ident_a = const.tile([P, P], ADT, name="ident_a")
                make_identity(nc, ident_a[:])
            else:
                ident_a = ident
            mskp_sb = const.tile([P, NMT], F32, name="mskp_sb")
            nc.sync.dma_start(mskp_sb[:], mskp_h[:].rearrange("(t p) -> p t", p=P))
            # PE warmup: dummy matmuls fill the DMA-bound startup window so the
            # HAM clock gate reaches 8/8 before real matmuls begin
            for _ in range(150):
                ps_w = pacc.tile([P, P], F32, tag="pmm", name="ps_warm")
                nc.tensor.matmul(out=ps_w[:], lhsT=rr(ident_a[:]),
                                 rhs=rr(ident_a[:]), start=True, stop=True)
            ones_k = const.tile([P, 1], ADT, name="ones_k")
            nc.vector.memset(ones_k[:], 1.0)
            ones_b = const.tile([1, P], ADT, name="ones_b")
            nc.vector.memset(ones_b[:], 1.0)
            if not use_bf16:
                nc.vector.tensor_copy(out=ro(ones_k[:]), in_=ones_k[:])
                nc.vector.tensor_copy(out=ro(ones_b[:]), in_=ones_b[:])
            eps_1 = const.tile([1, 1], F32, name="eps_1")
            nc.vector.memset(eps_1[:], EPS)
            eps_p = const.tile([P, 1], F32, name="eps_p")
            nc.vector.memset(eps_p[:], EPS)

            pmv = const.tile([P, OHR], ADT, name="pmv")
            pooled = const.tile([P, DB, mc_pad], ADT, name="pooled")
            pooled_rm = const.tile([P, NMT, D], ADT, name="pooled_rm")
            nc.vector.memset(pooled_rm[:], 0.0)
            nc.vector.tensor_copy(out=ra(pooled_rm[:]), in_=pooled_rm[:])

            # ---- prologue: vp matmul per gather-tile ----
            for t in range(NMT):
                xs, xe = gx[t]
                ps_pmv = pacc.tile([P, P], F32, tag="pmm")
                for fb in range(12):
                    src = xs if fb < 6 else xe
                    blk = src[:, (fb % 6) * P:(fb % 6 + 1) * P]
                    ps_t = pacc.tile([P, P], F32, tag="pmm")
                    nc.tensor.transpose(out=ps_t[:], in_=blk, identity=ident[:])
                    xb = wblk.tile([P, P], ADT, tag="blk")
                    nc.scalar.copy(out=ra(xb[:]), in_=ps_t[:])
                    nc.tensor.matmul(
                        out=ps_pmv[:], lhsT=rr(w_vp[:, fb, :]), rhs=rr(xb[:]),
                        start=(fb == 0), stop=(fb == 11))
                nc.scalar.activation(
                    out=ra(pmv[:, t * P:(t + 1) * P]), in_=ps_pmv[:],
                    func=AF.Identity, bias=vp_b[:, 0:1])

            # ---- main loop over mention chunks (LN phase pipelined 1 back) ----
            def emit_stats(st):
                """LN stats + per-row coefficient chain for a finished chunk."""
                c, zt, scv = st
                ps_s1 = pst.tile([1, NF], F32, tag="s1")
                ps_s2 = pst.tile([1, NF], F32, tag="s2")
                for db in range(DB):
                    nc.tensor.matmul(
                        out=ps_s1[:], lhsT=rr(ones_k[:]), rhs=rr(zt[db][:]),
                        start=(db == 0), stop=(db == DB - 1))
                    sq = w512.tile([P, NF], ADT, tag="sq", bufs=4)
                    nc.vector.tensor_tensor(out=ra(sq[:]), in0=zt[db][:],
                                            in1=zt[db][:], op=OP.mult)
                    nc.tensor.matmul(
                        out=ps_s2[:], lhsT=rr(ones_k[:]), rhs=rr(sq[:]),
                        start=(db == 0), stop=(db == DB - 1))
                mu = sm.tile([1, NF], F32, tag="st5", bufs=5)
                nc.scalar.activation(out=mu[:], in_=ps_s1[:], func=AF.Copy,
                                     scale=1.0 / D)
                musq = sm.tile([1, NF], F32, tag="st5", bufs=5)
                nc.vector.tensor_tensor(out=musq[:], in0=mu[:], in1=mu[:],
                                        op=OP.mult)
                var = sm.tile([1, NF], F32, tag="st5", bufs=5)
                nc.vector.scalar_tensor_tensor(
                    out=var[:], in0=ps_s2[:], scalar=1.0 / D, in1=musq[:],
                    op0=OP.mult, op1=OP.subtract)
                nc.scalar.activation(out=var[:], in_=var[:], func=AF.Sqrt,
                                     bias=eps_1[:])
                rstd = sm.tile([1, NF], F32, tag="st5", bufs=5)
                nc.vector.reciprocal_approx_fast(out=rstd[:], in_=var[:])

                if aclb_zero:
                    # fold: A*score = (z*c1 - c0), gamma applied post-reduce
                    c1f = sm.tile([1, NF], F32, tag="st5", bufs=5)
                    nc.vector.tensor_tensor(out=c1f[:], in0=rstd[:], in1=scv[:],
                                            op=OP.mult)
                    c1 = sm.tile([1, NF], ADT, tag="rstd", bufs=(4 if aclb_zero else 8))
                    with nc.allow_low_precision(reason="matmul rhs rounding"):
                        nc.vector.tensor_copy(out=ra(c1[:]), in_=c1f[:])
                    c0f = sm.tile([1, NF], F32, tag="st5", bufs=5)
                    nc.vector.tensor_tensor(out=c0f[:], in0=mu[:],
                                            in1=c1f[:], op=OP.mult)
                    s0a = sm.tile([1, CH], ADT, tag="s0", bufs=3)
                    with nc.allow_low_precision(reason="matmul rhs rounding"):
                        nc.vector.tensor_reduce(
                            out=ra(s0a[:]),
                            in_=c0f[:].rearrange("p (m k) -> p m k", k=K),
                            axis=AX.X, op=OP.add)
                    return (c, zt, scv, c1, s0a, None, None)
                rstd_a = sm.tile([1, NF], ADT, tag="rstd", bufs=(4 if aclb_zero else 8))
                with nc.allow_low_precision(reason="matmul rhs rounding"):
                    nc.vector.tensor_copy(out=ra(rstd_a[:]), in_=rstd[:])
                mu_a = sm.tile([1, NF], ADT, tag="rstd", bufs=(4 if aclb_zero else 8))
                with nc.allow_low_precision(reason="matmul rhs rounding"):
                    nc.vector.tensor_copy(out=ra(mu_a[:]), in_=mu[:])
                sc_a = sm.tile([1, NF], ADT, tag="rstd", bufs=(4 if aclb_zero else 8))
                with nc.allow_low_precision(reason="matmul rhs rounding"):
                    nc.vector.tensor_copy(out=ra(sc_a[:]), in_=scv[:])
                return (c, zt, scv, mu_a, rstd_a, sc_a, None)

            def emit_apply(st2):
                """Broadcast + LN apply + pooling; input from emit_stats."""
                if aclb_zero:
                    c, zt, scv, c1, s0a, _, _ = st2
                    ms = slice(c * CH, (c + 1) * CH)
                    c1_b = pb.tile([P, NF], F32, tag="bc")
                    nc.tensor.matmul(out=c1_b[:], lhsT=rr(ones_b[:]),
                                     rhs=rr(c1[:]), start=True, stop=True)
                    s0_b = pb.tile([P, CH], F32, tag="bc")
                    nc.tensor.matmul(out=s0_b[:], lhsT=rr(ones_b[:]),
                                     rhs=rr(s0a[:]), start=True, stop=True)
                    for db in range(DB):
                        wv = w512.tile([P, NF], F32, tag="wv", bufs=3)
                        nc.vector.tensor_tensor(out=wv[:], in0=zt[db][:],
                                                in1=c1_b[:], op=OP.mult)
                        with nc.allow_low_precision(reason="matmul rhs rounding"):
                            nc.vector.tensor_reduce(
                                out=ra(pooled[:, db, ms]),
                                in_=wv[:].rearrange("p (m k) -> p m k", k=K),
                                axis=AX.X, op=OP.add)
                        with nc.allow_low_precision(reason="pool correction bf16"):
                            nc.vector.tensor_tensor(
                                out=pooled[:, db, ms], in0=pooled[:, db, ms],
                                in1=s0_b[:], op=OP.subtract)
                            nc.vector.tensor_scalar_mul(
                                out=pooled[:, db, ms], in0=pooled[:, db, ms],
                                scalar1=ac_ls[:, db:db + 1])
                else:
                    c, zt, scv, mu_a, rstd_a, sc_a, _ = st2
                    ms = slice(c * CH, (c + 1) * CH)
                    mu_b = pb.tile([P, NF], F32, tag="bc")
                    nc.tensor.matmul(out=mu_b[:], lhsT=rr(ones_b[:]),
                                     rhs=rr(mu_a[:]), start=True, stop=True)
                    r_b = pb.tile([P, NF], F32, tag="bc")
                    nc.tensor.matmul(out=r_b[:], lhsT=rr(ones_b[:]),
                                     rhs=rr(rstd_a[:]), start=True, stop=True)
                    sc_b = pb.tile([P, NF], F32, tag="bc")
                    nc.tensor.matmul(out=sc_b[:], lhsT=rr(ones_b[:]),
                                     rhs=rr(sc_a[:]), start=True, stop=True)
                    for db in range(DB):
                        wv = w512.tile([P, NF], F32, tag="wv", bufs=3)
                        nc.vector.tensor_tensor(out=wv[:], in0=zt[db][:],
                                                in1=mu_b[:], op=OP.subtract)
                        nc.vector.tensor_tensor(out=wv[:], in0=wv[:], in1=r_b[:],
                                                op=OP.mult)
                        nc.vector.tensor_scalar(
                            out=wv[:], in0=wv[:], scalar1=ac_ls[:, db:db + 1],
                            scalar2=ac_lb[:, db:db + 1], op0=OP.mult, op1=OP.add)
                        nc.vector.tensor_tensor(out=wv[:], in0=wv[:], in1=sc_b[:],
                                                op=OP.mult)
                        with nc.allow_low_precision(reason="matmul rhs rounding"):
                            nc.vector.tensor_reduce(
                                out=ra(pooled[:, db, ms]),
                                in_=wv[:].rearrange("p (m k) -> p m k", k=K),
                                axis=AX.X, op=OP.add)

            pending = None
            for c in range(NCH):
                cs, ce = c * NF, (c + 1) * NF
                ms = slice(c * CH, (c + 1) * CH)
                if c < NPRE:
                    rv_c = rv_pre[c]
                else:
                    rv_c = rvp.tile([P, NF], WDT, tag="rv", bufs=5)
                    nc.sync.dma_start(rv_c[:], rvt_h[:, cs:ce])
                scv = sm.tile([1, NF], F32, tag="scv", bufs=3)
                nc.sync.dma_start(scv[:], sco_h[:, cs:ce])

                # cm: h1 = gelu(cm_top^T @ pmv_bcast + cm_bot^T @ rv + cm_b)
                pmv_bc = pmv[:, ms][:, :, None].to_broadcast((P, CH, K))
                h1 = act8.tile([P, HB, NF], FP8 if use_fp8 else ADT,
                               tag="h1", bufs=2)
                for hb in range(HB):
                    ps = pacc.tile([P, NF], F32, tag="pmm")
                    nc.tensor.matmul(
                        out=ps[:], lhsT=rr(w_cm[:, 0, hb * P:(hb + 1) * P]),
                        rhs=rr(pmv_bc), start=True, stop=False)
                    nc.tensor.matmul(
                        out=ps[:], lhsT=rr(w_cm[:, 1, hb * P:(hb + 1) * P]),
                        rhs=rr(rv_c[:]), start=False, stop=True)
                    with nc.allow_low_precision(reason="fp8 activations"):
                        nc.scalar.activation(
                            out=h1[:, hb, :], in_=ps[:], func=GELU,
                            bias=cm_b[:, hb:hb + 1])

                if pending is not None:
                    pend_stats = emit_stats(pending)

                # cd: x2 = cd^T @ h1 + cd_b  (fp8 DoubleRow)
                x2 = act8.tile([P, DB, NF], FP8 if use_fp8 else ADT,
                               tag="x2", bufs=2)
                for db in range(DB):
                    ps = pacc.tile([P, NF], F32, tag="pmm")
                    if use_fp8:
                        for q in range(HB // 2):
                            nc.tensor.matmul(
                                out=ps[:],
                                lhsT=w_cd[:, 2 * q:2 * q + 2, db * P:(db + 1) * P],
                                rhs=h1[:, 2 * q:2 * q + 2, :],
                                perf_mode=DR,
                                start=(q == 0), stop=(q == HB // 2 - 1))
                    else:
                        for kb in range(HB):
                            nc.tensor.matmul(
                                out=ps[:], lhsT=rr(w_cd[:, kb, db * P:(db + 1) * P]),
                                rhs=rr(h1[:, kb, :]), start=(kb == 0),
                                stop=(kb == HB - 1))
                    with nc.allow_low_precision(reason="fp8 activations"):
                        nc.scalar.activation(
                            out=x2[:, db, :], in_=ps[:], func=AF.Identity,
                            scale=1.0 / s_cd, bias=cd_b[:, db:db + 1])

                if pending is not None:
                    emit_apply(pend_stats)
                    pending = None

                # ac1: h2 = gelu(a1^T @ x2 + a1_b)  (fp8 DoubleRow)
                h2 = act8.tile([P, HB, NF], FP8 if use_fp8 else ADT,
                               tag="h2", bufs=2)
                for hb in range(HB):
                    ps = pacc.tile([P, NF], F32, tag="pmm")
                    if use_fp8:
                        for q in range(DB // 2):
                            nc.tensor.matmul(
                                out=ps[:],
                                lhsT=w_a1[:, 2 * q:2 * q + 2, hb * P:(hb + 1) * P],
                                rhs=x2[:, 2 * q:2 * q + 2, :],
                                perf_mode=DR,
                                start=(q == 0), stop=(q == DB // 2 - 1))
                    else:
                        for kb in range(DB):
                            nc.tensor.matmul(
                                out=ps[:], lhsT=rr(w_a1[:, kb, hb * P:(hb + 1) * P]),
                                rhs=rr(x2[:, kb, :]), start=(kb == 0),
                                stop=(kb == DB - 1))
                    with nc.allow_low_precision(reason="fp8 activations"):
                        nc.scalar.activation(
                            out=h2[:, hb, :], in_=ps[:], func=GELU,
                            scale=1.0 / s_a1, bias=a1_b[:, hb:hb + 1])

                # ac2 + residual: z = a2^T @ h2 / s + a2_b + x2
                zt = []
                for db in range(DB):
                    ps = pacc.tile([P, NF], F32, tag="pmm")
                    if use_fp8:
                        for q in range(HB // 2):
                            nc.tensor.matmul(
                                out=ps[:],
                                lhsT=w_a2[:, 2 * q:2 * q + 2, db * P:(db + 1) * P],
                                rhs=h2[:, 2 * q:2 * q + 2, :],
                                perf_mode=DR,
                                start=(q == 0), stop=(q == HB // 2 - 1))
                    else:
                        for kb in range(HB):
                            nc.tensor.matmul(
                                out=ps[:], lhsT=rr(w_a2[:, kb, db * P:(db + 1) * P]),
                                rhs=rr(h2[:, kb, :]), start=(kb == 0),
                                stop=(kb == HB - 1))
                    zb = w512.tile([P, NF], ADT, tag="z", bufs=13)
                    with nc.allow_low_precision(reason="fp8 residual"):
                        nc.vector.scalar_tensor_tensor(
                            out=ra(zb[:]), in0=ps[:], scalar=1.0 / s_a2,
                            in1=x2[:, db, :], op0=OP.mult, op1=OP.add)
                    if not b2_zero:
                        nc.vector.tensor_scalar_add(
                            out=ra(zb[:]), in0=zb[:],
                            scalar1=a2_b[:, db:db + 1])
                    zt.append(zb)

                pending = (c, zt, scv)
                if c == 1:
                    w_p1 = const.tile([P, DB, H], WDT, name="w_p1")
                    nc.sync.dma_start(
                        w_p1[:], p1w_h[:].rearrange("(kb p) m -> p kb m", p=P))
                    w_p2 = const.tile([P, HB, D], WDT, name="w_p2")
                    nc.sync.dma_start(
                        w_p2[:], p2w_h[:].rearrange("(kb p) m -> p kb m", p=P))
                    oh_sb = const.tile([P, NMT, NTT, P], WDT, name="oh_sb")
                    nc.sync.dma_start(
                        oh_sb[:],
                        oh_h[:].rearrange("(mt p) (tt q) -> p mt tt q", p=P, q=P))
            st2 = emit_stats(pending)
            emit_apply(st2)

            # ---- pl MLP on pooled [D, mc_pad] ----
            g1 = []
            for hb in range(HB):
                ps = pacc.tile([P, mc_pad], F32, tag="pmm")
                for db in range(DB):
                    nc.tensor.matmul(
                        out=ps[:], lhsT=rr(w_p1[:, db, hb * P:(hb + 1) * P]),
                        rhs=rr(pooled[:, db, :]),
                        start=(db == 0), stop=(db == DB - 1))
                gb = w512.tile([P, mc_pad], ADT, tag="h1g", bufs=9)
                nc.scalar.activation(out=ra(gb[:]), in_=ps[:], func=GELU,
                                     bias=p1_b[:, hb:hb + 1])
                g1.append(gb)
            zp = []
            for db in range(DB):
                ps = pacc.tile([P, mc_pad], F32, tag="pmm")
                for kb in range(HB):
                    nc.tensor.matmul(
                        out=ps[:], lhsT=rr(w_p2[:, kb, db * P:(db + 1) * P]),
                        rhs=rr(g1[kb][:]),
                        start=(kb == 0), stop=(kb == HB - 1))
                zb = w512.tile([P, mc_pad], ADT, tag="z", bufs=13)
                nc.vector.scalar_tensor_tensor(
                    out=ra(zb[:]), in0=ps[:], scalar=p2_b[:, db:db + 1],
                    in1=pooled[:, db, :], op0=OP.add, op1=OP.add)
                zp.append(zb)

            # pl LayerNorm + mask
            if pl_triv:
                # transpose zp to row-major first (overlaps pl2 on PE), then
                # LayerNorm each 128-mention block row-major via ACT accum
                for db in range(DB):
                    for mt in range(NMT):
                        npart = min(P, mc_pad - mt * P)
                        ps_t = pacc.tile([P, P], ADT, tag="pmm")
                        nc.tensor.transpose(
                            out=ps_t[:npart, :],
                            in_=zp[db][:, mt * P:mt * P + npart],
                            identity=ident_a[:])
                        nc.scalar.copy(
                            out=ra(pooled_rm[:npart, mt, db * P:(db + 1) * P]),
                            in_=ps_t[:npart, :])
                for mt in range(NMT):
                    rmv = pooled_rm[:, mt, :]
                    scr = w768.tile([P, D], F32, tag="scr", bufs=4)
                    mean = sm.tile([P, 1], F32, tag="bnmv", bufs=8)
                    nc.scalar.activation(out=scr[:], in_=rmv, func=AF.Copy,
                                         scale=1.0 / D, accum_out=mean[:])
                    ez2 = sm.tile([P, 1], F32, tag="bnmv", bufs=8)
                    nc.scalar.activation(out=scr[:], in_=rmv, func=AF.Square,
                                         scale=1.0 / math.sqrt(D),
                                         accum_out=ez2[:])
                    nv = sm.tile([P, 1], F32, tag="bnmv", bufs=8)
                    nc.vector.scalar_tensor_tensor(
                        out=nv[:], in0=mean[:], scalar=mean[:], in1=ez2[:],
                        op0=OP.mult, op1=OP.subtract)
                    nc.scalar.activation(out=nv[:], in_=nv[:], func=AF.Sqrt,
                                         scale=-1.0, bias=eps_p[:])
                    nc.vector.reciprocal_approx_fast(out=nv[:], in_=nv[:])
                    with nc.allow_low_precision(reason="bf16 pooled apply"):
                        nc.vector.tensor_scalar(
                            out=rmv, in0=rmv, scalar1=mean[:], scalar2=nv[:],
                            op0=OP.subtract, op1=OP.mult)
                        nc.vector.tensor_scalar_mul(
                            out=rmv, in0=rmv, scalar1=mskp_sb[:, mt:mt + 1])
            else:
                ps_s1 = pst.tile([1, mc_pad], F32, tag="s1")
                ps_s2 = pst.tile([1, mc_pad], F32, tag="s2")
                for db in range(DB):
                    nc.tensor.matmul(out=ps_s1[:], lhsT=rr(ones_k[:]),
                                     rhs=rr(zp[db][:]),
                                     start=(db == 0), stop=(db == DB - 1))
                    sq = w512.tile([P, mc_pad], ADT, tag="sq", bufs=4)
                    nc.vector.tensor_tensor(out=ra(sq[:]), in0=zp[db][:],
                                            in1=zp[db][:], op=OP.mult)
                    nc.tensor.matmul(out=ps_s2[:], lhsT=rr(ones_k[:]),
                                     rhs=rr(sq[:]),
                                     start=(db == 0), stop=(db == DB - 1))
                mu = sm.tile([1, NF], F32, tag="st5", bufs=5,
                             name="plmu")[:, :mc_pad]
                nc.scalar.activation(out=mu, in_=ps_s1[:], func=AF.Copy,
                                     scale=1.0 / D)
                musq = sm.tile([1, NF], F32, tag="st5", bufs=5,
                               name="plmusq")[:, :mc_pad]
                nc.vector.tensor_tensor(out=musq, in0=mu, in1=mu, op=OP.mult)
                var = sm.tile([1, NF], F32, tag="st5", bufs=5,
                              name="plvar")[:, :mc_pad]
                nc.vector.scalar_tensor_tensor(
                    out=var, in0=ps_s2[:], scalar=1.0 / D, in1=musq,
                    op0=OP.mult, op1=OP.subtract)
                nc.scalar.activation(out=var, in_=var, func=AF.Sqrt,
                                     bias=eps_1[:])
                rstd = sm.tile([1, NF], F32, tag="st5", bufs=5,
                               name="plrstdf")[:, :mc_pad]
                nc.vector.reciprocal_approx_fast(out=rstd, in_=var)
                mu_a = sm.tile([1, NF], ADT, tag="rstd", bufs=(4 if aclb_zero else 8),
                               name="plmua")[:, :mc_pad]
                with nc.allow_low_precision(reason="matmul rhs rounding"):
                    nc.vector.tensor_copy(out=ra(mu_a), in_=mu)
                rstd_a = sm.tile([1, NF], ADT, tag="rstd", bufs=(4 if aclb_zero else 8),
                                 name="plrstda")[:, :mc_pad]
                with nc.allow_low_precision(reason="matmul rhs rounding"):
                    nc.vector.tensor_copy(out=ra(rstd_a), in_=rstd)
                mu_b = pb.tile([P, mc_pad], F32, tag="bc")
                nc.tensor.matmul(out=mu_b[:], lhsT=rr(ones_b[:]), rhs=rr(mu_a),
                                 start=True, stop=True)
                r_b = pb.tile([P, mc_pad], F32, tag="bc")
                nc.tensor.matmul(out=r_b[:], lhsT=rr(ones_b[:]),
                                 rhs=rr(rstd_a), start=True, stop=True)
                mk_a = sm.tile([1, NF], ADT, tag="rstd", bufs=(4 if aclb_zero else 8),
                               name="plmka")[:, :mc_pad]
                with nc.allow_low_precision(reason="matmul rhs rounding"):
                    nc.vector.tensor_copy(out=ra(mk_a), in_=msk_sb[:])
                mk_b = pb.tile([P, mc_pad], F32, tag="bc")
                nc.tensor.matmul(out=mk_b[:], lhsT=rr(ones_b[:]), rhs=rr(mk_a),
                                 start=True, stop=True)
                for db in range(DB):
                    pf = w512.tile([P, mc_pad], F32, tag="x2f", bufs=7)
                    nc.vector.tensor_tensor(out=pf[:], in0=zp[db][:],
                                            in1=mu_b[:], op=OP.subtract)
                    nc.vector.tensor_tensor(out=pf[:], in0=pf[:], in1=r_b[:],
                                            op=OP.mult)
                    nc.vector.tensor_scalar(
                        out=pf[:], in0=pf[:], scalar1=pl_ls[:, db:db + 1],
                        scalar2=pl_lb[:, db:db + 1], op0=OP.mult, op1=OP.add)
                    nc.vector.tensor_tensor(out=pf[:], in0=pf[:], in1=mk_b[:],
                                            op=OP.mult)
                    for mt in range(NMT):
                        npart = min(P, mc_pad - mt * P)
                        ps_t = pacc.tile([P, P], F32, tag="pmm")
                        nc.tensor.transpose(
                            out=ps_t[:npart, :],
                            in_=pf[:, mt * P:mt * P + npart],
                            identity=ident[:])
                        nc.scalar.copy(
                            out=ra(pooled_rm[:npart, mt, db * P:(db + 1) * P]),
                            in_=ps_t[:npart, :])

            # ---- scatter via one-hot matmul + final row-major LayerNorm ----
            for tt in range(NTT):
                e_t = w768.tile([P, D], F32, tag="g768")
                nc.gpsimd.dma_start(e_t[:], enc_h[tt * P:(tt + 1) * P, :])
                ps_d1 = pacc.tile([P, 512], F32, tag="pmm")
                ps_d2 = pb.tile([P, 256], F32, tag="bc")
                for mt in range(NMT):
                    nc.tensor.matmul(
                        out=ps_d1[:], lhsT=rr(oh_sb[:, mt, tt, :]),
                        rhs=rr(pooled_rm[:, mt, 0:512]),
                        start=(mt == 0), stop=(mt == NMT - 1))
                    nc.tensor.matmul(
                        out=ps_d2[:], lhsT=rr(oh_sb[:, mt, tt, :]),
                        rhs=rr(pooled_rm[:, mt, 512:768]),
                        start=(mt == 0), stop=(mt == NMT - 1))
                z_t = w768.tile([P, D], F32, tag="g768")
                nc.vector.tensor_tensor(out=z_t[:, 0:512], in0=e_t[:, 0:512],
                                        in1=ps_d1[:], op=OP.add)
                nc.vector.tensor_tensor(out=z_t[:, 512:768], in0=e_t[:, 512:768],
                                        in1=ps_d2[:], op=OP.add)
                # row stats via ACT free-dim accumulate: mean and E[z^2]
                scr = w768.tile([P, D], F32, tag="scr", bufs=4)
                mean = sm.tile([P, 1], F32, tag="bnmv", bufs=8)
                nc.scalar.activation(out=scr[:], in_=z_t[:], func=AF.Copy,
                                     scale=1.0 / D, accum_out=mean[:])
                ez2 = sm.tile([P, 1], F32, tag="bnmv", bufs=8)
                nc.scalar.activation(out=scr[:], in_=z_t[:], func=AF.Square,
                                     scale=1.0 / math.sqrt(D), accum_out=ez2[:])
                nv = sm.tile([P, 1], F32, tag="bnmv", bufs=8)
                nc.vector.scalar_tensor_tensor(
                    out=nv[:], in0=mean[:], scalar=mean[:], in1=ez2[:],
                    op0=OP.mult, op1=OP.subtract)
                nc.scalar.activation(out=nv[:], in_=nv[:], func=AF.Sqrt,
                                     scale=-1.0, bias=eps_p[:])
                nc.vector.reciprocal_approx_fast(out=nv[:], in_=nv[:])
                nc.vector.tensor_scalar(
                    out=z_t[:], in0=z_t[:], scalar1=mean[:],
                    scalar2=nv[:], op0=OP.subtract, op1=OP.mult)
                if not ln_triv:
                    nc.vector.tensor_tensor(out=z_t[:], in0=z_t[:],
                                            in1=lns_rep[:], op=OP.mult)
                    nc.vector.tensor_tensor(out=z_t[:], in0=z_t[:],
                                            in1=lnb_rep[:], op=OP.add)
                nc.sync.dma_start(out_h[tt * P:(tt + 1) * P, :], z_t[:])

    nc.compile()
    return nc


_CACHE = {}


def _flags():
    return (os.environ.get("KB_F32R", "1") == "1",
            os.environ.get("KB_TANH_GELU", "1") == "1",
            os.environ.get("KB_BF16", "1") == "1",
            os.environ.get("KB_FP8", "1") == "1")


def _get_program(mc_pad, scales, aclb_zero=True, b2_zero=True, pl_triv=True,
                 ln_triv=True):
    key = (mc_pad, scales, aclb_zero, b2_zero, pl_triv, ln_triv) + _flags()
    if key not in _CACHE:
        f32r, tanh, bf16, fp8 = _flags()
        _CACHE[key] = _build(mc_pad, scales, f32r, tanh, bf16, fp8, aclb_zero,
                             b2_zero, pl_triv, ln_triv)
    return _CACHE[key]


def _pack_rows(counts):
    """Partition 32 batch rows into 8 groups of 4, minimizing max group sum."""
    counts = np.asarray(counts)
    rng = np.random.default_rng(12345)
    best = None
    target = int(np.ceil(counts.sum() / NCORES))
    for _ in range(64):
        perm = rng.permutation(B)
        groups = perm.reshape(NCORES, BPC)
        sums = counts[groups].sum(1)
        for _ in range(400):
            mx = sums.max()
            i = int(sums.argmax())
            j = int(sums.argmin())
            done = False
            for a in range(BPC):
                for b in range(BPC):
                    d = counts[groups[i, a]] - counts[groups[j, b]]
                    if d > 0 and max(sums[i] - d, sums[j] + d) < mx:
                        groups[i, a], groups[j, b] = groups[j, b], groups[i, a]
                        sums[i] -= d
                        sums[j] += d
                        done = True
                        break
                if done:
                    break
            if not done:
                break
        m = int(sums.max())
        if best is None or m < best[0]:
            best = (m, groups.copy())
        if best[0] <= target:
            break
    return best[1]


def _host_prep(inputs, bf16=True):
    import ml_dtypes
    wt = ml_dtypes.bfloat16 if bf16 else np.float32
    enc = np.ascontiguousarray(np.asarray(inputs['encoded_input'], np.float32))
    rv = np.asarray(inputs['retrieval_values'], np.float32)
    sc = np.asarray(inputs['retrieval_scores'], np.float32)
    mbp = np.asarray(inputs['mention_batch_positions']).astype(np.int64)
    msp = np.asarray(inputs['mention_start_positions']).astype(np.int64)
    mep = np.asarray(inputs['mention_end_positions']).astype(np.int64)
    mmask = np.asarray(inputs['mention_mask'], np.float32)

    row_counts = np.bincount(mbp, minlength=B)
    groups = _pack_rows(row_counts)          # [NCORES, BPC] original row ids
    owner_of_row = np.empty(B, np.int64)
    local_of_row = np.empty(B, np.int64)
    for c in range(NCORES):
        for j in range(BPC):
            owner_of_row[groups[c, j]] = c
            local_of_row[groups[c, j]] = j
    owner = owner_of_row[mbp]
    counts = np.bincount(owner, minlength=NCORES)
    mc_pad = int(np.max(counts))
    mc_pad = max(CH, ((mc_pad + CH - 1) // CH) * CH)
    if mc_pad > 512:
        raise ValueError(f"mc_pad {mc_pad} > 512 unsupported")
    nmt = (mc_pad + P - 1) // P
    ohr = nmt * P

    cores = []
    for c in range(NCORES):
        ids = np.nonzero(owner == c)[0]
        n = len(ids)
        d = dict(
            enc=np.ascontiguousarray(enc[groups[c]]).reshape(TOK, D),
            gis=np.zeros(ohr, np.int32), gie=np.zeros(ohr, np.int32),
            rvt=np.zeros((R, mc_pad * K), wt),
            sco=np.zeros((1, mc_pad * K), np.float32),
            msk=np.zeros((1, mc_pad), wt),
            mskp=np.zeros(ohr, np.float32),
            oh=np.zeros((ohr, TOK), wt))
        if n:
            lb = local_of_row[mbp[ids]]
            slots = (lb * T + msp[ids]).astype(np.int64)
            d['gis'][:n] = slots.astype(np.int32)
            d['gie'][:n] = (lb * T + mep[ids]).astype(np.int32)
            d['rvt'][:, :n * K] = rv[ids].reshape(n * K, R).T.astype(wt)
            d['sco'][0, :n * K] = sc[ids].reshape(n * K)
            d['msk'][0, :n] = mmask[ids]
            d['mskp'][:n] = mmask[ids]
            d['oh'][np.arange(n), slots] = 1.0
        cores.append(d)
    return cores, mc_pad, groups


def _host_weights(inputs, bf16=True, fp8=True):
    import ml_dtypes
    wt = ml_dtypes.bfloat16 if bf16 else np.float32
    f8 = ml_dtypes.float8_e4m3
    f = lambda k: np.ascontiguousarray(np.asarray(inputs[k], np.float32))
    fw = lambda k: np.ascontiguousarray(np.asarray(inputs[k], np.float32).astype(wt))
    fm = lambda k, nb: np.ascontiguousarray(
        np.asarray(inputs[k], np.float32).reshape(nb, P).T)

    def q8(k):
        w = np.asarray(inputs[k], np.float32)
        s = 2.0 ** np.floor(np.log2(240.0 / max(np.abs(w).max(), 1e-30)))
        return np.ascontiguousarray((w * s).astype(f8)), float(s)

    if fp8:
        cdw, s_cd = q8('cd_w')
        a1w, s_a1 = q8('ac_w1')
        a2w, s_a2 = q8('ac_w2')
    else:
        cdw, s_cd = fw('cd_w'), 1.0
        a1w, s_a1 = fw('ac_w1'), 1.0
        a2w, s_a2 = fw('ac_w2'), 1.0
    w = dict(
        vpw=fw('vp_w'), vpb=fm('vp_b', 1),
        cmw=fw('cm_w'), cmb=fm('cm_b', HB),
        cdw=cdw, cdb=fm('cd_b', DB),
        a1w=a1w, a1b=fm('ac_b1', HB),
        a2w=a2w, a2b=fm('ac_b2', DB),
        acls=fm('ac_ln_s', DB), aclb=fm('ac_ln_b', DB),
        p1w=fw('pl_w1'), p1b=fm('pl_b1', HB),
        p2w=fw('pl_w2'), p2b=fm('pl_b2', DB),
        plls=fm('pl_ln_s', DB), pllb=fm('pl_ln_b', DB),
        lns=f('ln_s').reshape(1, D), lnb=f('ln_b').reshape(1, D))
    return w, (s_cd, s_a1, s_a2)


def _prepare(inputs):
    flags = _flags()
    bf16, fp8 = flags[2], flags[3]
    cores, mc_pad, groups = _host_prep(inputs, bf16)
    w, scales = _host_weights(inputs, bf16, fp8)
    aclb_zero = not np.any(np.asarray(inputs['ac_ln_b'], np.float32))
    b2_zero = not np.any(np.asarray(inputs['ac_b2'], np.float32))
    pl_triv = (np.all(np.asarray(inputs['pl_ln_s'], np.float32) == 1.0)
               and not np.any(np.asarray(inputs['pl_ln_b'], np.float32)))
    ln_triv = (np.all(np.asarray(inputs['ln_s'], np.float32) == 1.0)
               and not np.any(np.asarray(inputs['ln_b'], np.float32)))
    nc = _get_program(mc_pad, scales, bool(aclb_zero), bool(b2_zero),
                      bool(pl_triv), bool(ln_triv))
    in_maps = [{**w, **cd} for cd in cores]
    return nc, in_maps, groups


def kernel(**inputs):
    nc, in_maps, groups = _prepare(inputs)
    res = run_bass_kernel_spmd(nc, in_maps, core_ids=list(range(NCORES)))
    out = np.empty((B, T, D), np.float32)
    for c in range(NCORES):
        out[groups[c]] = res.results[c]['out'].reshape(BPC, T, D)
    return out


if __name__ == '__main__':
    import reference
    import jax
    with jax.default_device(jax.devices('cpu')[0]):
        jin = reference.setup_inputs()
        expected = np.asarray(reference.reference(**jin))
    inputs = {k: np.asarray(v) for k, v in jin.items()}
    actual = kernel(**inputs)
    rel = np.linalg.norm(actual - expected) / np.linalg.norm(expected)
    print('rel err:', rel)


# revision 20
# speedup vs baseline: 1.0583x; 1.0583x over previous
"""Trainium2 Bass kernel for nn_ConcatMLPUpdate (gnn_message_passing).

Strategy (8 NeuronCores, SPMD):
  - Bin-pack the 32 batch rows into 8 groups of 4 so each core owns ~256
    mentions (scatter-add back into encoded_input stays core-local); the
    output rows are un-permuted on the host.
  - All heavy activations are feature-major ([features<=128 on partitions,
    rows on the free dim]) so every matmul consumes natural-layout weights as
    the stationary operand and no transposes appear in the hot loop.
  - The three large matmuls (cd, ac1, ac2) run in fp8e4m3 with DoubleRow
    perf mode (2 contraction tiles per pass -> 2x PE rate). Weights are
    pre-scaled by a power of two on the host; the dequant rides the ACT/DVE
    evacuation scale.
  - The per-mention projection term enters the cm PSUM accumulation via a
    k-broadcast matmul of pmv (no per-chunk DVE broadcast add, no T1 buffer).
  - LayerNorm over the feature (partition) dim uses ones-vector matmuls for
    sum/sum-of-squares and a rank-1 ones matmul to broadcast per-row stats
    back across partitions.
  - The scatter-add is a one-hot matmul (handles duplicate target slots
    exactly), fused with the final row-major LayerNorm pass.

kernel(**inputs) takes the full unsharded inputs and returns the full output.
"""

import math
import os
import sys

import numpy as np

for _p in ("/opt/trn_rl_repo", "/root/.axon_site/_ro/trn_rl_repo"):
    if os.path.isdir(_p) and _p not in sys.path:
        sys.path.append(_p)

import concourse.bass as bass
from concourse import bacc
import concourse.tile as tile
from concourse import mybir
from concourse.bass import IndirectOffsetOnAxis
from concourse.bass_utils import run_bass_kernel_spmd
from concourse.masks import make_identity

# problem constants
B, T, D = 32, 512, 768
M, K, R = 2048, 32, 128
H = 1024
EPS = 1e-12
NCORES = 8
BPC = B // NCORES            # batch rows per core
TOK = BPC * T                # token slots per core
P = 128
CH = 16                      # mentions per main-loop chunk
NF = CH * K                  # free-dim columns per chunk (512)
DB = D // P                  # 6 feature blocks of D
HB = H // P                  # 8 feature blocks of H

F32 = mybir.dt.float32
F32R = mybir.dt.float32r
BF16 = mybir.dt.bfloat16
FP8 = mybir.dt.float8e4
I32 = mybir.dt.int32
AF = mybir.ActivationFunctionType
OP = mybir.AluOpType
AX = mybir.AxisListType
DR = mybir.MatmulPerfMode.DoubleRow


def _build(mc_pad, scales, use_f32r=True, use_tanh_gelu=True, use_bf16=True,
           use_fp8=True, aclb_zero=True, b2_zero=True, acls_one=True,
           pl_triv=True, ln_triv=True):
    """Build the Bass program for a padded per-core mention count."""
    NCH = mc_pad // CH
    NMT = (mc_pad + P - 1) // P      # 128-mention blocks (gather/scatter)
    OHR = NMT * P                    # one-hot row count (mc_pad padded to 128)
    NR = mc_pad * K                  # retrieval rows per core
    NTT = TOK // P                   # token tiles (16)
    GELU = AF.Gelu_apprx_tanh if use_tanh_gelu else AF.Gelu
    s_cd, s_a1, s_a2 = scales

    nc = bacc.Bacc("TRN2", target_bir_lowering=False, debug=False)

    def rr(ap):
        if ap.dtype != F32:
            return ap
        return ap.bitcast(F32R) if use_f32r else ap

    def ro(ap):
        # producer out-AP cast: ACT/DVE round their output to f32r precision
        return ap.bitcast(F32R) if use_f32r else ap

    WDT = BF16 if use_bf16 else (F32R if use_f32r else F32)
    ADT = BF16 if use_bf16 else F32
    MDT = FP8 if use_fp8 else WDT    # dtype of the three big weight sets

    def ra(ap):
        # round/cast for matmul-input activations produced by ACT/DVE
        if use_bf16:
            return ap
        return ro(ap)

    # ---- I/O ----
    dp = nc.declare_dram_parameter
    enc_h = dp("enc", [TOK, D], F32, isOutput=False)
    rvt_h = dp("rvt", [R, NR], WDT, isOutput=False)
    sco_h = dp("sco", [1, NR], F32, isOutput=False)
    msk_h = dp("msk", [1, mc_pad], WDT, isOutput=False)
    mskp_h = dp("mskp", [OHR], F32, isOutput=False)
    gis_h = dp("gis", [OHR], I32, isOutput=False)
    gie_h = dp("gie", [OHR], I32, isOutput=False)
    oh_h = dp("oh", [OHR, TOK], WDT, isOutput=False)
    vpw_h = dp("vpw", [2 * D, R], WDT, isOutput=False)
    vpb_h = dp("vpb", [P, 1], F32, isOutput=False)
    cmw_h = dp("cmw", [2 * R, H], WDT, isOutput=False)
    cmb_h = dp("cmb", [P, HB], F32, isOutput=False)
    cdw_h = dp("cdw", [H, D], MDT, isOutput=False)
    cdb_h = dp("cdb", [P, DB], F32, isOutput=False)
    a1w_h = dp("a1w", [D, H], MDT, isOutput=False)
    a1b_h = dp("a1b", [P, HB], F32, isOutput=False)
    a2w_h = dp("a2w", [H, D], MDT, isOutput=False)
    a2b_h = dp("a2b", [P, DB], F32, isOutput=False)
    acls_h = dp("acls", [P, DB], F32, isOutput=False)
    aclb_h = dp("aclb", [P, DB], F32, isOutput=False)
    p1w_h = dp("p1w", [D, H], WDT, isOutput=False)
    p1b_h = dp("p1b", [P, HB], F32, isOutput=False)
    p2w_h = dp("p2w", [H, D], WDT, isOutput=False)
    p2b_h = dp("p2b", [P, DB], F32, isOutput=False)
    plls_h = dp("plls", [P, DB], F32, isOutput=False)
    pllb_h = dp("pllb", [P, DB], F32, isOutput=False)
    lns_h = dp("lns", [1, D], F32, isOutput=False)
    lnb_h = dp("lnb", [1, D], F32, isOutput=False)
    out_h = dp("out", [TOK, D], F32, isOutput=True)

    with tile.TileContext(nc) as tc:
        with (
            tc.tile_pool(name="const", bufs=1) as const,
            tc.tile_pool(name="w512", bufs=2) as w512,
            tc.tile_pool(name="w768", bufs=7) as w768,
            tc.tile_pool(name="rvp", bufs=5) as rvp,
            tc.tile_pool(name="wblk", bufs=4) as wblk,
            tc.tile_pool(name="act8", bufs=2) as act8,
            tc.tile_pool(name="sm", bufs=2) as sm,
            tc.tile_pool(name="pacc", bufs=4 if aclb_zero else 3, space="PSUM") as pacc,
            tc.tile_pool(name="pb", bufs=2 if aclb_zero else 3, space="PSUM") as pb,
            tc.tile_pool(name="pst", bufs=1, space="PSUM") as pst,
        ):
            # ---- index DMAs + endpoint gathers first (long pole) ----
            gis_sb = const.tile([P, NMT], I32, name="gis_sb")
            nc.sync.dma_start(gis_sb[:], gis_h[:].rearrange("(t p) -> p t", p=P))
            gie_sb = const.tile([P, NMT], I32, name="gie_sb")
            nc.sync.dma_start(gie_sb[:], gie_h[:].rearrange("(t p) -> p t", p=P))
            gx = []
            for t in range(NMT):
                xs = w768.tile([P, D], F32, tag="g768", name=f"xs{t}")
                nc.gpsimd.indirect_dma_start(
                    out=xs[:], out_offset=None, in_=enc_h[:],
                    in_offset=IndirectOffsetOnAxis(ap=gis_sb[:, t:t + 1], axis=0))
                xe = w768.tile([P, D], F32, tag="g768", name=f"xe{t}")
                nc.gpsimd.indirect_dma_start(
                    out=xe[:], out_offset=None, in_=enc_h[:],
                    in_offset=IndirectOffsetOnAxis(ap=gie_sb[:, t:t + 1], axis=0))
                gx.append((xs, xe))

            # ---- persistent tiles (ordered so early-needed data lands first) ----
            w_vp = const.tile([P, 12, P], WDT, name="w_vp")
            nc.sync.dma_start(w_vp[:], vpw_h[:].rearrange("(kb p) m -> p kb m", p=P))
            w_cm = const.tile([P, 2, H], WDT, name="w_cm")
            nc.sync.dma_start(w_cm[:], cmw_h[:].rearrange("(kb p) m -> p kb m", p=P))
            NPRE = min(4, NCH)
            rv_pre = []
            for c in range(NPRE):
                rv_t = rvp.tile([P, NF], WDT, tag="rv", bufs=5, name=f"rvp{c}")
                nc.sync.dma_start(rv_t[:], rvt_h[:, c * NF:(c + 1) * NF])
                rv_pre.append(rv_t)
            def bias_tile(h, cols, nm):
                t = const.tile([P, cols], F32, name=nm)
                nc.sync.dma_start(t[:], h[:])
                return t

            vp_b = bias_tile(vpb_h, 1, "vp_b")
            cm_b = bias_tile(cmb_h, HB, "cm_b")
            cd_b = bias_tile(cdb_h, DB, "cd_b")
            a1_b = bias_tile(a1b_h, HB, "a1_b")
            a2_b = bias_tile(a2b_h, DB, "a2_b")
            ac_ls = bias_tile(acls_h, DB, "ac_ls")
            ac_lb = bias_tile(aclb_h, DB, "ac_lb")
            p1_b = bias_tile(p1b_h, HB, "p1_b")
            p2_b = bias_tile(p2b_h, DB, "p2_b")
            pl_ls = bias_tile(plls_h, DB, "pl_ls")
            pl_lb = bias_tile(pllb_h, DB, "pl_lb")

            w_cd = const.tile([P, HB, D], MDT, name="w_cd")
            nc.sync.dma_start(w_cd[:], cdw_h[:].rearrange("(kb p) m -> p kb m", p=P))
            w_a1 = const.tile([P, DB, H], MDT, name="w_a1")
            nc.sync.dma_start(w_a1[:], a1w_h[:].rearrange("(kb p) m -> p kb m", p=P))
            w_a2 = const.tile([P, HB, D], MDT, name="w_a2")
            nc.sync.dma_start(w_a2[:], a2w_h[:].rearrange("(kb p) m -> p kb m", p=P))

            if not ln_triv:
                lns_rep = const.tile([P, D], F32, name="lns_rep")
                nc.sync.dma_start(lns_rep[:], lns_h[:].to_broadcast((P, D)))
                lnb_rep = const.tile([P, D], F32, name="lnb_rep")
                nc.sync.dma_start(lnb_rep[:], lnb_h[:].to_broadcast((P, D)))

            msk_sb = const.tile([1, mc_pad], WDT, name="msk_sb")
            nc.sync.dma_start(msk_sb[:], msk_h[:])

            ident = const.tile([P, P], F32, name="ident")
            make_identity(nc, ident[:])
            if ADT != F32:
                ident_a = const.tile([P, P], ADT, name="ident_a")
                make_identity(nc, ident_a[:])
            else:
                ident_a = ident
            mskp_sb = const.tile([P, NMT], F32, name="mskp_sb")
            nc.sync.dma_start(mskp_sb[:], mskp_h[:].rearrange("(t p) -> p t", p=P))
            # PE warmup: dummy matmuls fill the DMA-bound startup window so the
            # HAM clock gate reaches 8/8 before real matmuls begin
            for _ in range(150):
                ps_w = pacc.tile([P, P], F32, tag="pmm", name="ps_warm")
                nc.tensor.matmul(out=ps_w[:], lhsT=rr(ident_a[:]),
                                 rhs=rr(ident_a[:]), start=True, stop=True)
            ones_k = const.tile([P, 1], ADT, name="ones_k")
            nc.vector.memset(ones_k[:], 1.0)
            ones_b = const.tile([1, P], ADT, name="ones_b")
            nc.vector.memset(ones_b[:], 1.0)
            if not use_bf16:
                nc.vector.tensor_copy(out=ro(ones_k[:]), in_=ones_k[:])
                nc.vector.tensor_copy(out=ro(ones_b[:]), in_=ones_b[:])
            eps_1 = const.tile([1, 1], F32, name="eps_1")
            nc.vector.memset(eps_1[:], EPS)
            eps_p = const.tile([P, 1], F32, name="eps_p")
            nc.vector.memset(eps_p[:], EPS)
            if use_fp8:
                ones_8 = const.tile([P, 2, P], FP8, name="ones_8")
                nc.vector.memset(ones_8[:], 1.0)

            pmv = const.tile([P, OHR], ADT, name="pmv")
            pooled = const.tile([P, DB, mc_pad], ADT, name="pooled")
            pooled_rm = const.tile([P, NMT, D], ADT, name="pooled_rm")
            nc.vector.memset(pooled_rm[:], 0.0)
            nc.vector.tensor_copy(out=ra(pooled_rm[:]), in_=pooled_rm[:])

            # ---- prologue: vp matmul per gather-tile ----
            for t in range(NMT):
                xs, xe = gx[t]
                ps_pmv = pacc.tile([P, P], F32, tag="pmm")
                for fb in range(12):
                    src = xs if fb < 6 else xe
                    blk = src[:, (fb % 6) * P:(fb % 6 + 1) * P]
                    ps_t = pacc.tile([P, P], F32, tag="pmm")
                    nc.tensor.transpose(out=ps_t[:], in_=blk, identity=ident[:])
                    xb = wblk.tile([P, P], ADT, tag="blk")
                    nc.scalar.copy(out=ra(xb[:]), in_=ps_t[:])
                    nc.tensor.matmul(
                        out=ps_pmv[:], lhsT=rr(w_vp[:, fb, :]), rhs=rr(xb[:]),
                        start=(fb == 0), stop=(fb == 11))
                nc.scalar.activation(
                    out=ra(pmv[:, t * P:(t + 1) * P]), in_=ps_pmv[:],
                    func=AF.Identity, bias=vp_b[:, 0:1])

            # ---- main loop over mention chunks (LN phase pipelined 1 back) ----
            def emit_stats(st):
                """LN stats + per-row coefficient chain for a finished chunk."""
                c, zt, scv = st
                ps_s1 = pst.tile([1, NF], F32, tag="s1")
                if use_fp8:
                    # sum of squares via fp8 DoubleRow: sq = (z/16)*z in fp8,
                    # paired feature blocks contract two-at-a-time. The ones
                    # lhsT is [P,2,P] (every PSUM row gets the sum; row 0 used)
                    ps_s2 = pst.tile([P, NF], F32, tag="s2")
                    sq = act8.tile([P, DB, NF], FP8, tag="sq", bufs=2)
                    for db in range(DB):
                        nc.tensor.matmul(
                            out=ps_s1[:], lhsT=rr(ones_k[:]), rhs=rr(zt[db][:]),
                            start=(db == 0), stop=(db == DB - 1))
                        with nc.allow_low_precision(reason="fp8 sq"):
                            nc.vector.scalar_tensor_tensor(
                                out=sq[:, db, :], in0=zt[db][:], scalar=1.0 / 16,
                                in1=zt[db][:], op0=OP.mult, op1=OP.mult)
                    for q in range(DB // 2):
                        nc.tensor.matmul(
                            out=ps_s2[:], lhsT=ones_8[:],
                            rhs=sq[:, 2 * q:2 * q + 2, :], perf_mode=DR,
                            start=(q == 0), stop=(q == DB // 2 - 1))
                    s2row = ps_s2[0:1, :]
                    s2scale = 16.0 / D
                else:
                    ps_s2 = pst.tile([1, NF], F32, tag="s2")
                    for db in range(DB):
                        nc.tensor.matmul(
                            out=ps_s1[:], lhsT=rr(ones_k[:]), rhs=rr(zt[db][:]),
                            start=(db == 0), stop=(db == DB - 1))
                        sq = w512.tile([P, NF], ADT, tag="sq", bufs=4)
                        nc.vector.tensor_tensor(out=ra(sq[:]), in0=zt[db][:],
                                                in1=zt[db][:], op=OP.mult)
                        nc.tensor.matmul(
                            out=ps_s2[:], lhsT=rr(ones_k[:]), rhs=rr(sq[:]),
                            start=(db == 0), stop=(db == DB - 1))
                    s2row = ps_s2[:]
                    s2scale = 1.0 / D
                mu = sm.tile([1, NF], F32, tag="st5", bufs=5)
                nc.scalar.activation(out=mu[:], in_=ps_s1[:], func=AF.Copy,
                                     scale=1.0 / D)
                musq = sm.tile([1, NF], F32, tag="st5", bufs=5)
                nc.vector.tensor_tensor(out=musq[:], in0=mu[:], in1=mu[:],
                                        op=OP.mult)
                var = sm.tile([1, NF], F32, tag="st5", bufs=5)
                nc.vector.scalar_tensor_tensor(
                    out=var[:], in0=s2row, scalar=s2scale, in1=musq[:],
                    op0=OP.mult, op1=OP.subtract)
                nc.scalar.activation(out=var[:], in_=var[:], func=AF.Sqrt,
                                     bias=eps_1[:])
                rstd = sm.tile([1, NF], F32, tag="st5", bufs=5)
                nc.vector.reciprocal_approx_fast(out=rstd[:], in_=var[:])

                if aclb_zero:
                    # fold: A*score = (z*c1 - c0), gamma applied post-reduce
                    c1f = sm.tile([1, NF], F32, tag="st5", bufs=5)
                    nc.vector.tensor_tensor(out=c1f[:], in0=rstd[:], in1=scv[:],
                                            op=OP.mult)
                    c1 = sm.tile([1, NF], ADT, tag="rstd", bufs=(4 if aclb_zero else 8))
                    with nc.allow_low_precision(reason="matmul rhs rounding"):
                        nc.vector.tensor_copy(out=ra(c1[:]), in_=c1f[:])
                    c0f = sm.tile([1, NF], F32, tag="st5", bufs=5)
                    nc.vector.tensor_tensor(out=c0f[:], in0=mu[:],
                                            in1=c1f[:], op=OP.mult)
                    s0a = sm.tile([1, CH], ADT, tag="s0", bufs=3)
                    with nc.allow_low_precision(reason="matmul rhs rounding"):
                        nc.vector.tensor_reduce(
                            out=ra(s0a[:]),
                            in_=c0f[:].rearrange("p (m k) -> p m k", k=K),
                            axis=AX.X, op=OP.add)
                    return (c, zt, scv, c1, s0a, None, None)
                rstd_a = sm.tile([1, NF], ADT, tag="rstd", bufs=(4 if aclb_zero else 8))
                with nc.allow_low_precision(reason="matmul rhs rounding"):
                    nc.vector.tensor_copy(out=ra(rstd_a[:]), in_=rstd[:])
                mu_a = sm.tile([1, NF], ADT, tag="rstd", bufs=(4 if aclb_zero else 8))
                with nc.allow_low_precision(reason="matmul rhs rounding"):
                    nc.vector.tensor_copy(out=ra(mu_a[:]), in_=mu[:])
                sc_a = sm.tile([1, NF], ADT, tag="rstd", bufs=(4 if aclb_zero else 8))
                with nc.allow_low_precision(reason="matmul rhs rounding"):
                    nc.vector.tensor_copy(out=ra(sc_a[:]), in_=scv[:])
                return (c, zt, scv, mu_a, rstd_a, sc_a, None)

            def emit_apply(st2):
                """Broadcast + LN apply + pooling; input from emit_stats."""
                if aclb_zero:
                    c, zt, scv, c1, s0a, _, _ = st2
                    ms = slice(c * CH, (c + 1) * CH)
                    c1_b = pb.tile([P, NF], F32, tag="bc")
                    nc.tensor.matmul(out=c1_b[:], lhsT=rr(ones_b[:]),
                                     rhs=rr(c1[:]), start=True, stop=True)
                    s0_b = pb.tile([P, CH], F32, tag="bc")
                    nc.tensor.matmul(out=s0_b[:], lhsT=rr(ones_b[:]),
                                     rhs=rr(s0a[:]), start=True, stop=True)
                    for db in range(DB):
                        wv = w512.tile([P, NF], F32, tag="wv", bufs=3)
                        nc.vector.tensor_tensor(out=wv[:], in0=zt[db][:],
                                                in1=c1_b[:], op=OP.mult)
                        with nc.allow_low_precision(reason="matmul rhs rounding"):
                            nc.vector.tensor_reduce(
                                out=ra(pooled[:, db, ms]),
                                in_=wv[:].rearrange("p (m k) -> p m k", k=K),
                                axis=AX.X, op=OP.add)
                        with nc.allow_low_precision(reason="pool correction bf16"):
                            nc.vector.tensor_tensor(
                                out=pooled[:, db, ms], in0=pooled[:, db, ms],
                                in1=s0_b[:], op=OP.subtract)
                            if not acls_one:
                                nc.vector.tensor_scalar_mul(
                                    out=pooled[:, db, ms], in0=pooled[:, db, ms],
                                    scalar1=ac_ls[:, db:db + 1])
                else:
                    c, zt, scv, mu_a, rstd_a, sc_a, _ = st2
                    ms = slice(c * CH, (c + 1) * CH)
                    mu_b = pb.tile([P, NF], F32, tag="bc")
                    nc.tensor.matmul(out=mu_b[:], lhsT=rr(ones_b[:]),
                                     rhs=rr(mu_a[:]), start=True, stop=True)
                    r_b = pb.tile([P, NF], F32, tag="bc")
                    nc.tensor.matmul(out=r_b[:], lhsT=rr(ones_b[:]),
                                     rhs=rr(rstd_a[:]), start=True, stop=True)
                    sc_b = pb.tile([P, NF], F32, tag="bc")
                    nc.tensor.matmul(out=sc_b[:], lhsT=rr(ones_b[:]),
                                     rhs=rr(sc_a[:]), start=True, stop=True)
                    for db in range(DB):
                        wv = w512.tile([P, NF], F32, tag="wv", bufs=3)
                        nc.vector.tensor_tensor(out=wv[:], in0=zt[db][:],
                                                in1=mu_b[:], op=OP.subtract)
                        nc.vector.tensor_tensor(out=wv[:], in0=wv[:], in1=r_b[:],
                                                op=OP.mult)
                        nc.vector.tensor_scalar(
                            out=wv[:], in0=wv[:], scalar1=ac_ls[:, db:db + 1],
                            scalar2=ac_lb[:, db:db + 1], op0=OP.mult, op1=OP.add)
                        nc.vector.tensor_tensor(out=wv[:], in0=wv[:], in1=sc_b[:],
                                                op=OP.mult)
                        with nc.allow_low_precision(reason="matmul rhs rounding"):
                            nc.vector.tensor_reduce(
                                out=ra(pooled[:, db, ms]),
                                in_=wv[:].rearrange("p (m k) -> p m k", k=K),
                                axis=AX.X, op=OP.add)

            pending = None
            for c in range(NCH):
                cs, ce = c * NF, (c + 1) * NF
                ms = slice(c * CH, (c + 1) * CH)
                if c < NPRE:
                    rv_c = rv_pre[c]
                else:
                    rv_c = rvp.tile([P, NF], WDT, tag="rv", bufs=5)
                    nc.sync.dma_start(rv_c[:], rvt_h[:, cs:ce])
                scv = sm.tile([1, NF], F32, tag="scv", bufs=3)
                nc.sync.dma_start(scv[:], sco_h[:, cs:ce])

                # cm: h1 = gelu(cm_top^T @ pmv_bcast + cm_bot^T @ rv + cm_b)
                pmv_bc = pmv[:, ms][:, :, None].to_broadcast((P, CH, K))
                h1 = act8.tile([P, HB, NF], FP8 if use_fp8 else ADT,
                               tag="h1", bufs=2)
                for hb in range(HB):
                    ps = pacc.tile([P, NF], F32, tag="pmm")
                    nc.tensor.matmul(
                        out=ps[:], lhsT=rr(w_cm[:, 0, hb * P:(hb + 1) * P]),
                        rhs=rr(pmv_bc), start=True, stop=False)
                    nc.tensor.matmul(
                        out=ps[:], lhsT=rr(w_cm[:, 1, hb * P:(hb + 1) * P]),
                        rhs=rr(rv_c[:]), start=False, stop=True)
                    with nc.allow_low_precision(reason="fp8 activations"):
                        nc.scalar.activation(
                            out=h1[:, hb, :], in_=ps[:], func=GELU,
                            bias=cm_b[:, hb:hb + 1])

                if pending is not None:
                    pend_stats = emit_stats(pending)

                # cd: x2 = cd^T @ h1 + cd_b  (fp8 DoubleRow)
                x2 = act8.tile([P, DB, NF], FP8 if use_fp8 else ADT,
                               tag="x2", bufs=2)
                for db in range(DB):
                    ps = pacc.tile([P, NF], F32, tag="pmm")
                    if use_fp8:
                        for q in range(HB // 2):
                            nc.tensor.matmul(
                                out=ps[:],
                                lhsT=w_cd[:, 2 * q:2 * q + 2, db * P:(db + 1) * P],
                                rhs=h1[:, 2 * q:2 * q + 2, :],
                                perf_mode=DR,
                                start=(q == 0), stop=(q == HB // 2 - 1))
                    else:
                        for kb in range(HB):
                            nc.tensor.matmul(
                                out=ps[:], lhsT=rr(w_cd[:, kb, db * P:(db + 1) * P]),
                                rhs=rr(h1[:, kb, :]), start=(kb == 0),
                                stop=(kb == HB - 1))
                    with nc.allow_low_precision(reason="fp8 activations"):
                        nc.scalar.activation(
                            out=x2[:, db, :], in_=ps[:], func=AF.Identity,
                            scale=1.0 / s_cd, bias=cd_b[:, db:db + 1])

                if pending is not None:
                    emit_apply(pend_stats)
                    pending = None

                # ac1: h2 = gelu(a1^T @ x2 + a1_b)  (fp8 DoubleRow)
                h2 = act8.tile([P, HB, NF], FP8 if use_fp8 else ADT,
                               tag="h2", bufs=2)
                for hb in range(HB):
                    ps = pacc.tile([P, NF], F32, tag="pmm")
                    if use_fp8:
                        for q in range(DB // 2):
                            nc.tensor.matmul(
                                out=ps[:],
                                lhsT=w_a1[:, 2 * q:2 * q + 2, hb * P:(hb + 1) * P],
                                rhs=x2[:, 2 * q:2 * q + 2, :],
                                perf_mode=DR,
                                start=(q == 0), stop=(q == DB // 2 - 1))
                    else:
                        for kb in range(DB):
                            nc.tensor.matmul(
                                out=ps[:], lhsT=rr(w_a1[:, kb, hb * P:(hb + 1) * P]),
                                rhs=rr(x2[:, kb, :]), start=(kb == 0),
                                stop=(kb == DB - 1))
                    with nc.allow_low_precision(reason="fp8 activations"):
                        nc.scalar.activation(
                            out=h2[:, hb, :], in_=ps[:], func=GELU,
                            scale=1.0 / s_a1, bias=a1_b[:, hb:hb + 1])

                # ac2 + residual: z = a2^T @ h2 / s + a2_b + x2
                zt = []
                for db in range(DB):
                    ps = pacc.tile([P, NF], F32, tag="pmm")
                    if use_fp8:
                        for q in range(HB // 2):
                            nc.tensor.matmul(
                                out=ps[:],
                                lhsT=w_a2[:, 2 * q:2 * q + 2, db * P:(db + 1) * P],
                                rhs=h2[:, 2 * q:2 * q + 2, :],
                                perf_mode=DR,
                                start=(q == 0), stop=(q == HB // 2 - 1))
                    else:
                        for kb in range(HB):
                            nc.tensor.matmul(
                                out=ps[:], lhsT=rr(w_a2[:, kb, db * P:(db + 1) * P]),
                                rhs=rr(h2[:, kb, :]), start=(kb == 0),
                                stop=(kb == HB - 1))
                    zb = w512.tile([P, NF], ADT, tag="z", bufs=13)
                    if b2_zero:
                        # evacuate PSUM on ACT (prompt, keeps PE unblocked),
                        # then add the residual from SBUF on DVE
                        z0 = w512.tile([P, NF], ADT, tag="z0", bufs=3)
                        with nc.allow_low_precision(reason="fp8 residual"):
                            nc.scalar.activation(
                                out=z0[:], in_=ps[:], func=AF.Copy,
                                scale=1.0 / s_a2)
                            nc.vector.tensor_tensor(
                                out=ra(zb[:]), in0=z0[:], in1=x2[:, db, :],
                                op=OP.add)
                    else:
                        with nc.allow_low_precision(reason="fp8 residual"):
                            nc.vector.scalar_tensor_tensor(
                                out=ra(zb[:]), in0=ps[:], scalar=1.0 / s_a2,
                                in1=x2[:, db, :], op0=OP.mult, op1=OP.add)
                        nc.vector.tensor_scalar_add(
                            out=ra(zb[:]), in0=zb[:],
                            scalar1=a2_b[:, db:db + 1])
                    zt.append(zb)

                pending = (c, zt, scv)
                if c == 1:
                    w_p1 = const.tile([P, DB, H], WDT, name="w_p1")
                    nc.sync.dma_start(
                        w_p1[:], p1w_h[:].rearrange("(kb p) m -> p kb m", p=P))
                    w_p2 = const.tile([P, HB, D], WDT, name="w_p2")
                    nc.sync.dma_start(
                        w_p2[:], p2w_h[:].rearrange("(kb p) m -> p kb m", p=P))
                    oh_sb = const.tile([P, NMT, NTT, P], WDT, name="oh_sb")
                    nc.sync.dma_start(
                        oh_sb[:],
                        oh_h[:].rearrange("(mt p) (tt q) -> p mt tt q", p=P, q=P))
            st2 = emit_stats(pending)
            emit_apply(st2)

            # prefetch the scatter-phase encoder tiles while pl runs
            enc_t = []
            for tt in range(NTT):
                e_t = w768.tile([P, D], F32, tag="enc", bufs=5)
                nc.gpsimd.dma_start(e_t[:], enc_h[tt * P:(tt + 1) * P, :])
                enc_t.append(e_t)

            # ---- pl MLP on pooled [D, mc_pad] ----
            g1 = []
            for hb in range(HB):
                ps = pacc.tile([P, mc_pad], F32, tag="pmm")
                for db in range(DB):
                    nc.tensor.matmul(
                        out=ps[:], lhsT=rr(w_p1[:, db, hb * P:(hb + 1) * P]),
                        rhs=rr(pooled[:, db, :]),
                        start=(db == 0), stop=(db == DB - 1))
                gb = w512.tile([P, mc_pad], ADT, tag="h1g", bufs=9)
                nc.scalar.activation(out=ra(gb[:]), in_=ps[:], func=GELU,
                                     bias=p1_b[:, hb:hb + 1])
                g1.append(gb)
            zp = []
            for db in range(DB):
                ps = pacc.tile([P, mc_pad], F32, tag="pmm")
                for kb in range(HB):
                    nc.tensor.matmul(
                        out=ps[:], lhsT=rr(w_p2[:, kb, db * P:(db + 1) * P]),
                        rhs=rr(g1[kb][:]),
                        start=(kb == 0), stop=(kb == HB - 1))
                zb = w512.tile([P, mc_pad], ADT, tag="z", bufs=13)
                nc.vector.scalar_tensor_tensor(
                    out=ra(zb[:]), in0=ps[:], scalar=p2_b[:, db:db + 1],
                    in1=pooled[:, db, :], op0=OP.add, op1=OP.add)
                zp.append(zb)

            # pl LayerNorm + mask
            if pl_triv:
                # transpose zp to row-major first (overlaps pl2 on PE), then
                # LayerNorm each 128-mention block row-major via ACT accum
                for db in range(DB):
                    for mt in range(NMT):
                        npart = min(P, mc_pad - mt * P)
                        ps_t = pacc.tile([P, P], ADT, tag="pmm")
                        nc.tensor.transpose(
                            out=ps_t[:npart, :],
                            in_=zp[db][:, mt * P:mt * P + npart],
                            identity=ident_a[:])
                        nc.scalar.copy(
                            out=ra(pooled_rm[:npart, mt, db * P:(db + 1) * P]),
                            in_=ps_t[:npart, :])
                for mt in range(NMT):
                    rmv = pooled_rm[:, mt, :]
                    scr = w768.tile([P, D], F32, tag="scr", bufs=4)
                    mean = sm.tile([P, 1], F32, tag="bnmv", bufs=8)
                    nc.scalar.activation(out=scr[:], in_=rmv, func=AF.Copy,
                                         scale=1.0 / D, accum_out=mean[:])
                    ez2 = sm.tile([P, 1], F32, tag="bnmv", bufs=8)
                    nc.scalar.activation(out=scr[:], in_=rmv, func=AF.Square,
                                         scale=1.0 / math.sqrt(D),
                                         accum_out=ez2[:])
                    nv = sm.tile([P, 1], F32, tag="bnmv", bufs=8)
                    nc.vector.scalar_tensor_tensor(
                        out=nv[:], in0=mean[:], scalar=mean[:], in1=ez2[:],
                        op0=OP.mult, op1=OP.subtract)
                    nc.scalar.activation(out=nv[:], in_=nv[:], func=AF.Sqrt,
                                         scale=-1.0, bias=eps_p[:])
                    nc.vector.reciprocal_approx_fast(out=nv[:], in_=nv[:])
                    with nc.allow_low_precision(reason="bf16 pooled apply"):
                        nc.vector.tensor_scalar(
                            out=rmv, in0=rmv, scalar1=mean[:], scalar2=nv[:],
                            op0=OP.subtract, op1=OP.mult)
                        nc.vector.tensor_scalar_mul(
                            out=rmv, in0=rmv, scalar1=mskp_sb[:, mt:mt + 1])
            else:
                ps_s1 = pst.tile([1, mc_pad], F32, tag="s1")
                ps_s2 = pst.tile([1, mc_pad], F32, tag="s2")
                for db in range(DB):
                    nc.tensor.matmul(out=ps_s1[:], lhsT=rr(ones_k[:]),
                                     rhs=rr(zp[db][:]),
                                     start=(db == 0), stop=(db == DB - 1))
                    sq = w512.tile([P, mc_pad], ADT, tag="sq", bufs=4)
                    nc.vector.tensor_tensor(out=ra(sq[:]), in0=zp[db][:],
                                            in1=zp[db][:], op=OP.mult)
                    nc.tensor.matmul(out=ps_s2[:], lhsT=rr(ones_k[:]),
                                     rhs=rr(sq[:]),
                                     start=(db == 0), stop=(db == DB - 1))
                mu = sm.tile([1, NF], F32, tag="st5", bufs=5,
                             name="plmu")[:, :mc_pad]
                nc.scalar.activation(out=mu, in_=ps_s1[:], func=AF.Copy,
                                     scale=1.0 / D)
                musq = sm.tile([1, NF], F32, tag="st5", bufs=5,
                               name="plmusq")[:, :mc_pad]
                nc.vector.tensor_tensor(out=musq, in0=mu, in1=mu, op=OP.mult)
                var = sm.tile([1, NF], F32, tag="st5", bufs=5,
                              name="plvar")[:, :mc_pad]
                nc.vector.scalar_tensor_tensor(
                    out=var, in0=ps_s2[:], scalar=1.0 / D, in1=musq,
                    op0=OP.mult, op1=OP.subtract)
                nc.scalar.activation(out=var, in_=var, func=AF.Sqrt,
                                     bias=eps_1[:])
                rstd = sm.tile([1, NF], F32, tag="st5", bufs=5,
                               name="plrstdf")[:, :mc_pad]
                nc.vector.reciprocal_approx_fast(out=rstd, in_=var)
                mu_a = sm.tile([1, NF], ADT, tag="rstd", bufs=(4 if aclb_zero else 8),
                               name="plmua")[:, :mc_pad]
                with nc.allow_low_precision(reason="matmul rhs rounding"):
                    nc.vector.tensor_copy(out=ra(mu_a), in_=mu)
                rstd_a = sm.tile([1, NF], ADT, tag="rstd", bufs=(4 if aclb_zero else 8),
                                 name="plrstda")[:, :mc_pad]
                with nc.allow_low_precision(reason="matmul rhs rounding"):
                    nc.vector.tensor_copy(out=ra(rstd_a), in_=rstd)
                mu_b = pb.tile([P, mc_pad], F32, tag="bc")
                nc.tensor.matmul(out=mu_b[:], lhsT=rr(ones_b[:]), rhs=rr(mu_a),
                                 start=True, stop=True)
                r_b = pb.tile([P, mc_pad], F32, tag="bc")
                nc.tensor.matmul(out=r_b[:], lhsT=rr(ones_b[:]),
                                 rhs=rr(rstd_a), start=True, stop=True)
                mk_a = sm.tile([1, NF], ADT, tag="rstd", bufs=(4 if aclb_zero else 8),
                               name="plmka")[:, :mc_pad]
                with nc.allow_low_precision(reason="matmul rhs rounding"):
                    nc.vector.tensor_copy(out=ra(mk_a), in_=msk_sb[:])
                mk_b = pb.tile([P, mc_pad], F32, tag="bc")
                nc.tensor.matmul(out=mk_b[:], lhsT=rr(ones_b[:]), rhs=rr(mk_a),
                                 start=True, stop=True)
                for db in range(DB):
                    pf = w512.tile([P, mc_pad], F32, tag="x2f", bufs=7)
                    nc.vector.tensor_tensor(out=pf[:], in0=zp[db][:],
                                            in1=mu_b[:], op=OP.subtract)
                    nc.vector.tensor_tensor(out=pf[:], in0=pf[:], in1=r_b[:],
                                            op=OP.mult)
                    nc.vector.tensor_scalar(
                        out=pf[:], in0=pf[:], scalar1=pl_ls[:, db:db + 1],
                        scalar2=pl_lb[:, db:db + 1], op0=OP.mult, op1=OP.add)
                    nc.vector.tensor_tensor(out=pf[:], in0=pf[:], in1=mk_b[:],
                                            op=OP.mult)
                    for mt in range(NMT):
                        npart = min(P, mc_pad - mt * P)
                        ps_t = pacc.tile([P, P], F32, tag="pmm")
                        nc.tensor.transpose(
                            out=ps_t[:npart, :],
                            in_=pf[:, mt * P:mt * P + npart],
                            identity=ident[:])
                        nc.scalar.copy(
                            out=ra(pooled_rm[:npart, mt, db * P:(db + 1) * P]),
                            in_=ps_t[:npart, :])

            # ---- scatter via one-hot matmul + final row-major LayerNorm ----
            for tt in range(NTT):
                e_t = enc_t[tt]
                ps_d1 = pacc.tile([P, 512], F32, tag="pmm")
                ps_d2 = pb.tile([P, 256], F32, tag="bc")
                for mt in range(NMT):
                    nc.tensor.matmul(
                        out=ps_d1[:], lhsT=rr(oh_sb[:, mt, tt, :]),
                        rhs=rr(pooled_rm[:, mt, 0:512]),
                        start=(mt == 0), stop=(mt == NMT - 1))
                    nc.tensor.matmul(
                        out=ps_d2[:], lhsT=rr(oh_sb[:, mt, tt, :]),
                        rhs=rr(pooled_rm[:, mt, 512:768]),
                        start=(mt == 0), stop=(mt == NMT - 1))
                z_t = w768.tile([P, D], F32, tag="zt", bufs=3)
                nc.vector.tensor_tensor(out=z_t[:, 0:512], in0=e_t[:, 0:512],
                                        in1=ps_d1[:], op=OP.add)
                nc.vector.tensor_tensor(out=z_t[:, 512:768], in0=e_t[:, 512:768],
                                        in1=ps_d2[:], op=OP.add)
                # row stats via ACT free-dim accumulate: mean and E[z^2]
                scr = w768.tile([P, D], F32, tag="scr", bufs=4)
                mean = sm.tile([P, 1], F32, tag="bnmv", bufs=8)
                nc.scalar.activation(out=scr[:], in_=z_t[:], func=AF.Copy,
                                     scale=1.0 / D, accum_out=mean[:])
                ez2 = sm.tile([P, 1], F32, tag="bnmv", bufs=8)
                nc.scalar.activation(out=scr[:], in_=z_t[:], func=AF.Square,
                                     scale=1.0 / math.sqrt(D), accum_out=ez2[:])
                nv = sm.tile([P, 1], F32, tag="bnmv", bufs=8)
                nc.vector.scalar_tensor_tensor(
                    out=nv[:], in0=mean[:], scalar=mean[:], in1=ez2[:],
                    op0=OP.mult, op1=OP.subtract)
                nc.scalar.activation(out=nv[:], in_=nv[:], func=AF.Sqrt,
                                     scale=-1.0, bias=eps_p[:])
                nc.vector.reciprocal_approx_fast(out=nv[:], in_=nv[:])
                nc.vector.tensor_scalar(
                    out=z_t[:], in0=z_t[:], scalar1=mean[:],
                    scalar2=nv[:], op0=OP.subtract, op1=OP.mult)
                if not ln_triv:
                    nc.vector.tensor_tensor(out=z_t[:], in0=z_t[:],
                                            in1=lns_rep[:], op=OP.mult)
                    nc.vector.tensor_tensor(out=z_t[:], in0=z_t[:],
                                            in1=lnb_rep[:], op=OP.add)
                nc.sync.dma_start(out_h[tt * P:(tt + 1) * P, :], z_t[:])

    nc.compile()
    return nc


_CACHE = {}


def _flags():
    return (os.environ.get("KB_F32R", "1") == "1",
            os.environ.get("KB_TANH_GELU", "1") == "1",
            os.environ.get("KB_BF16", "1") == "1",
            os.environ.get("KB_FP8", "1") == "1")


def _get_program(mc_pad, scales, aclb_zero=True, b2_zero=True, acls_one=True,
                 pl_triv=True, ln_triv=True):
    key = (mc_pad, scales, aclb_zero, b2_zero, acls_one, pl_triv,
           ln_triv) + _flags()
    if key not in _CACHE:
        f32r, tanh, bf16, fp8 = _flags()
        _CACHE[key] = _build(mc_pad, scales, f32r, tanh, bf16, fp8, aclb_zero,
                             b2_zero, acls_one, pl_triv, ln_triv)
    return _CACHE[key]


def _pack_rows(counts):
    """Partition 32 batch rows into 8 groups of 4, minimizing max group sum."""
    counts = np.asarray(counts)
    rng = np.random.default_rng(12345)
    best = None
    target = int(np.ceil(counts.sum() / NCORES))
    for _ in range(64):
        perm = rng.permutation(B)
        groups = perm.reshape(NCORES, BPC)
        sums = counts[groups].sum(1)
        for _ in range(400):
            mx = sums.max()
            i = int(sums.argmax())
            j = int(sums.argmin())
            done = False
            for a in range(BPC):
                for b in range(BPC):
                    d = counts[groups[i, a]] - counts[groups[j, b]]
                    if d > 0 and max(sums[i] - d, sums[j] + d) < mx:
                        groups[i, a], groups[j, b] = groups[j, b], groups[i, a]
                        sums[i] -= d
                        sums[j] += d
                        done = True
                        break
                if done:
                    break
            if not done:
                break
        m = int(sums.max())
        if best is None or m < best[0]:
            best = (m, groups.copy())
        if best[0] <= target:
            break
    return best[1]


def _host_prep(inputs, bf16=True):
    import ml_dtypes
    wt = ml_dtypes.bfloat16 if bf16 else np.float32
    enc = np.ascontiguousarray(np.asarray(inputs['encoded_input'], np.float32))
    rv = np.asarray(inputs['retrieval_values'], np.float32)
    sc = np.asarray(inputs['retrieval_scores'], np.float32)
    mbp = np.asarray(inputs['mention_batch_positions']).astype(np.int64)
    msp = np.asarray(inputs['mention_start_positions']).astype(np.int64)
    mep = np.asarray(inputs['mention_end_positions']).astype(np.int64)
    mmask = np.asarray(inputs['mention_mask'], np.float32)

    row_counts = np.bincount(mbp, minlength=B)
    groups = _pack_rows(row_counts)          # [NCORES, BPC] original row ids
    owner_of_row = np.empty(B, np.int64)
    local_of_row = np.empty(B, np.int64)
    for c in range(NCORES):
        for j in range(BPC):
            owner_of_row[groups[c, j]] = c
            local_of_row[groups[c, j]] = j
    owner = owner_of_row[mbp]
    counts = np.bincount(owner, minlength=NCORES)
    mc_pad = int(np.max(counts))
    mc_pad = max(CH, ((mc_pad + CH - 1) // CH) * CH)
    if mc_pad > 512:
        raise ValueError(f"mc_pad {mc_pad} > 512 unsupported")
    nmt = (mc_pad + P - 1) // P
    ohr = nmt * P

    cores = []
    for c in range(NCORES):
        ids = np.nonzero(owner == c)[0]
        n = len(ids)
        d = dict(
            enc=np.ascontiguousarray(enc[groups[c]]).reshape(TOK, D),
            gis=np.zeros(ohr, np.int32), gie=np.zeros(ohr, np.int32),
            rvt=np.zeros((R, mc_pad * K), wt),
            sco=np.zeros((1, mc_pad * K), np.float32),
            msk=np.zeros((1, mc_pad), wt),
            mskp=np.zeros(ohr, np.float32),
            oh=np.zeros((ohr, TOK), wt))
        if n:
            lb = local_of_row[mbp[ids]]
            slots = (lb * T + msp[ids]).astype(np.int64)
            d['gis'][:n] = slots.astype(np.int32)
            d['gie'][:n] = (lb * T + mep[ids]).astype(np.int32)
            d['rvt'][:, :n * K] = rv[ids].reshape(n * K, R).T.astype(wt)
            d['sco'][0, :n * K] = sc[ids].reshape(n * K)
            d['msk'][0, :n] = mmask[ids]
            d['mskp'][:n] = mmask[ids]
            d['oh'][np.arange(n), slots] = 1.0
        cores.append(d)
    return cores, mc_pad, groups


def _host_weights(inputs, bf16=True, fp8=True):
    import ml_dtypes
    wt = ml_dtypes.bfloat16 if bf16 else np.float32
    f8 = ml_dtypes.float8_e4m3
    f = lambda k: np.ascontiguousarray(np.asarray(inputs[k], np.float32))
    fw = lambda k: np.ascontiguousarray(np.asarray(inputs[k], np.float32).astype(wt))
    fm = lambda k, nb: np.ascontiguousarray(
        np.asarray(inputs[k], np.float32).reshape(nb, P).T)

    def q8(k):
        w = np.asarray(inputs[k], np.float32)
        s = 2.0 ** np.floor(np.log2(240.0 / max(np.abs(w).max(), 1e-30)))
        return np.ascontiguousarray((w * s).astype(f8)), float(s)

    if fp8:
        cdw, s_cd = q8('cd_w')
        a1w, s_a1 = q8('ac_w1')
        a2w, s_a2 = q8('ac_w2')
    else:
        cdw, s_cd = fw('cd_w'), 1.0
        a1w, s_a1 = fw('ac_w1'), 1.0
        a2w, s_a2 = fw('ac_w2'), 1.0
    w = dict(
        vpw=fw('vp_w'), vpb=fm('vp_b', 1),
        cmw=fw('cm_w'), cmb=fm('cm_b', HB),
        cdw=cdw, cdb=fm('cd_b', DB),
        a1w=a1w, a1b=fm('ac_b1', HB),
        a2w=a2w, a2b=fm('ac_b2', DB),
        acls=fm('ac_ln_s', DB), aclb=fm('ac_ln_b', DB),
        p1w=fw('pl_w1'), p1b=fm('pl_b1', HB),
        p2w=fw('pl_w2'), p2b=fm('pl_b2', DB),
        plls=fm('pl_ln_s', DB), pllb=fm('pl_ln_b', DB),
        lns=f('ln_s').reshape(1, D), lnb=f('ln_b').reshape(1, D))
    return w, (s_cd, s_a1, s_a2)


def _prepare(inputs):
    flags = _flags()
    bf16, fp8 = flags[2], flags[3]
    cores, mc_pad, groups = _host_prep(inputs, bf16)
    w, scales = _host_weights(inputs, bf16, fp8)
    aclb_zero = not np.any(np.asarray(inputs['ac_ln_b'], np.float32))
    b2_zero = not np.any(np.asarray(inputs['ac_b2'], np.float32))
    acls_one = np.all(np.asarray(inputs['ac_ln_s'], np.float32) == 1.0)
    pl_triv = (np.all(np.asarray(inputs['pl_ln_s'], np.float32) == 1.0)
               and not np.any(np.asarray(inputs['pl_ln_b'], np.float32)))
    ln_triv = (np.all(np.asarray(inputs['ln_s'], np.float32) == 1.0)
               and not np.any(np.asarray(inputs['ln_b'], np.float32)))
    nc = _get_program(mc_pad, scales, bool(aclb_zero), bool(b2_zero),
                      bool(acls_one), bool(pl_triv), bool(ln_triv))
    in_maps = [{**w, **cd} for cd in cores]
    return nc, in_maps, groups


def kernel(**inputs):
    nc, in_maps, groups = _prepare(inputs)
    res = run_bass_kernel_spmd(nc, in_maps, core_ids=list(range(NCORES)))
    out = np.empty((B, T, D), np.float32)
    for c in range(NCORES):
        out[groups[c]] = res.results[c]['out'].reshape(BPC, T, D)
    return out


if __name__ == '__main__':
    import reference
    import jax
    with jax.default_device(jax.devices('cpu')[0]):
        jin = reference.setup_inputs()
        expected = np.asarray(reference.reference(**jin))
    inputs = {k: np.asarray(v) for k, v in jin.items()}
    actual = kernel(**inputs)
    rel = np.linalg.norm(actual - expected) / np.linalg.norm(expected)
    print('rel err:', rel)
